# revision 9
# baseline (speedup 1.0000x reference)
"""Masked causal multi-head attention on 8 TRN2 NeuronCores.

Problem (hardcoded shapes): B=4, S=2048, D_MODEL=1024, HEADS=16,
KEY_SIZE=SIZE_PER_HEAD=64, OUT_DIM=1024, fp32 I/O.

Sharding: pure data/tensor parallel — core c handles batch b=c//2 and
head-group hg=c%2 (8 heads). Each core's output shard [2048, 512] is
independent, so there are no collectives; the host assembles shards.

Per-core pipeline:
  phase 1: DMA W shards + mask-bias constants into SBUF.
  phase 2: projections. qw^T/kw^T stored [dh, s] (bf16), vw stored
           [s, dh] (bf16) with a fused ones-column per head (65th col)
           so the PV matmul also produces softmax denominators.
  phase 3: per (head, q-half, k-tile): scores S^T[k, q] in one matmul
           (contraction over dh=64), causal bias add on the diagonal
           block, exp((S + causal)/8 + key_bias) on ScalarE (no
           max-subtraction: masked lanes underflow to exactly 0, same
           as the reference's -1e10 additive masks), PV matmul
           accumulates out^T [65, q] in PSUM (row 64 = sum of exp).
  host:    divide by sums, multiply q_mask, transpose, assemble.
"""

import os
import sys

import numpy as np

for _p in ("/opt/trn_rl_repo",):
    if _p not in sys.path and os.path.isdir(_p):
        sys.path.insert(0, _p)

import concourse.bass as bass
import concourse.mybir as mybir
import concourse.tile as tile
from concourse import bacc
from concourse.bass_utils import run_bass_kernel_spmd

B = 4
S = 2048
D = 1024
HEADS_PER_CORE = 8
DH = 64
HG_COLS = HEADS_PER_CORE * DH  # 512 output cols per core
NKT = S // 128  # 16 k-tiles
NEG = -1.0e9

F32 = mybir.dt.float32
BF16 = mybir.dt.bfloat16

LAST_RESULT = None  # stashed BassKernelResults for test harness inspection
_NC_CACHE = None


def _ensure_ntff_hook():
    """The agent image's antenv lacks axon_hooks; synthesize it so
    run_bass_kernel_spmd(trace=True) can reach the NTFF profiler."""
    try:
        from antenv.axon_hooks import get_axon_ntff_profile_hook  # noqa: F401

        return
    except ImportError:
        pass
    import types

    try:
        import antenv
        from trn_agent_boot.trn_boot import _ntff_profile_via_ctypes
    except ImportError:
        return
    mod = types.ModuleType("antenv.axon_hooks")
    _hook = [None]
    try:
        _hook[0] = _ntff_profile_via_ctypes("/opt/axon/libaxon_pjrt.so")
    except OSError:
        pass
    mod.set_axon_ntff_profile_hook = lambda h: _hook.__setitem__(0, h)
    mod.get_axon_ntff_profile_hook = lambda: _hook[0]
    sys.modules["antenv.axon_hooks"] = mod
    antenv.axon_hooks = mod


def _build_nc() -> bass.Bass:
    nc = bacc.Bacc()

    xqT = nc.declare_dram_parameter("xqT", [D, S], F32, isOutput=False)[:]
    xkT = nc.declare_dram_parameter("xkT", [D, S], F32, isOutput=False)[:]
    xvT = nc.declare_dram_parameter("xvT", [D, S], F32, isOutput=False)[:]
    wq = nc.declare_dram_parameter("wq", [D, HG_COLS], F32, isOutput=False)[:]
    wk = nc.declare_dram_parameter("wk", [D, HG_COLS], F32, isOutput=False)[:]
    wv = nc.declare_dram_parameter("wv", [D, HG_COLS], F32, isOutput=False)[:]
    vbias = nc.declare_dram_parameter("vbias", [128, NKT], F32, isOutput=False)[:]
    ctile = nc.declare_dram_parameter("ctile", [128, 128], F32, isOutput=False)[:]
    outT = nc.declare_dram_parameter(
        "outT", [HEADS_PER_CORE * 65, S], F32, isOutput=True
    )[:]

    with tile.TileContext(nc) as tc:
        with (
            tc.tile_pool(name="consts", bufs=1) as consts,
            tc.tile_pool(name="qk_sb", bufs=1) as qk_sb,
            tc.tile_pool(name="vw_pool", bufs=1) as vw_pool,
        ):
            vbias_sb = consts.tile([128, NKT], F32)
            nc.sync.dma_start(out=vbias_sb, in_=vbias)
            ctile_sb = consts.tile([128, 128], F32)
            nc.sync.dma_start(out=ctile_sb, in_=ctile)

            # projected tensors, resident for the whole attention phase
            qwT_sb = qk_sb.tile([128, 4, S], BF16)  # [dh%128, dh//128, s]
            kwT_sb = qk_sb.tile([128, 4, S], BF16)
            # vw with ones column: [s%128, s//128, head, 65]
            vw_sb = vw_pool.tile([128, NKT, HEADS_PER_CORE, 65], BF16)
            nc.vector.memset(vw_sb[:, :, :, 64:65], 1.0)

            # ---------------- phase 1+2: projections ----------------
            with (
                tc.tile_pool(name="wpool", bufs=1) as wpool,
                tc.tile_pool(name="xpool", bufs=2) as xpool,
                tc.tile_pool(name="pq", bufs=2, space="PSUM") as pqp,
                tc.tile_pool(name="pv", bufs=2, space="PSUM") as pvp,
            ):
                wq_sb = wpool.tile([128, 8, HG_COLS], F32, tag="wq")
                nc.sync.dma_start(out=wq_sb, in_=wq.rearrange("(t p) n -> p t n", p=128))
                wk_sb = wpool.tile([128, 8, HG_COLS], F32, tag="wk")
                nc.sync.dma_start(out=wk_sb, in_=wk.rearrange("(t p) n -> p t n", p=128))
                wv_sb = wpool.tile([128, 8, HG_COLS], F32, tag="wv")
                nc.sync.dma_start(out=wv_sb, in_=wv.rearrange("(t p) n -> p t n", p=128))

                xqT_r = xqT.rearrange("(t p) s -> p t s", p=128)
                xkT_r = xkT.rearrange("(t p) s -> p t s", p=128)
                xvT_r = xvT.rearrange("(t p) s -> p t s", p=128)

                SC = 256  # s-chunk width
                for sc in range(S // SC):
                    ssl = slice(sc * SC, (sc + 1) * SC)
                    xq_t = xpool.tile([128, 8, SC], F32, tag="xq")
                    nc.sync.dma_start(out=xq_t, in_=xqT_r[:, :, ssl])
                    xk_t = xpool.tile([128, 8, SC], F32, tag="xk")
                    nc.sync.dma_start(out=xk_t, in_=xkT_r[:, :, ssl])
                    xv_t = xpool.tile([128, 8, SC], F32, tag="xv")
                    nc.sync.dma_start(out=xv_t, in_=xvT_r[:, :, ssl])

                    # qw^T / kw^T: out[dh_tile, s] = W.T @ x^T
                    for w_sb, x_t, dst in (
                        (wq_sb, xq_t, qwT_sb),
                        (wk_sb, xk_t, kwT_sb),
                    ):
                        for dt in range(4):
                            ps = pqp.tile([128, SC], F32, tag="pq")
                            for t in range(8):
                                nc.tensor.matmul(
                                    ps,
                                    w_sb[:, t, dt * 128 : (dt + 1) * 128],
                                    x_t[:, t, :],
                                    start=(t == 0),
                                    stop=(t == 7),
                                )
                            nc.vector.tensor_copy(dst[:, dt, ssl], ps)

                    # vw: out[s_tile, dh] = x @ Wv  (natural layout)
                    for st2 in range(SC // 128):
                        kt = sc * (SC // 128) + st2
                        ps = pvp.tile([128, HG_COLS], F32, tag="pv")
                        for t in range(8):
                            nc.tensor.matmul(
                                ps,
                                xv_t[:, t, st2 * 128 : (st2 + 1) * 128],
                                wv_sb[:, t, :],
                                start=(t == 0),
                                stop=(t == 7),
                            )
                        nc.vector.tensor_copy(
                            vw_sb[:, kt, :, 0:64],
                            ps.rearrange("p (h d) -> p h d", h=HEADS_PER_CORE),
                        )

            # ---------------- phase 3: attention ----------------
            with (
                tc.tile_pool(name="ppool", bufs=3) as ppool,
                tc.tile_pool(name="ostage", bufs=2) as ostage,
                tc.tile_pool(name="att_s", bufs=2, space="PSUM") as spool,
                tc.tile_pool(name="att_o", bufs=2, space="PSUM") as opool,
            ):
                QH = 1024  # q processed in halves
                for h in range(HEADS_PER_CORE):
                    dt, poff = h // 2, (h % 2) * 64
                    for qh in range(S // QH):
                        q0 = qh * QH
                        out_ps = opool.tile([65, QH], F32, tag="outp")
                        last_kt = (q0 + QH) // 128 - 1
                        for kt in range(last_kt + 1):
                            q_off = max(0, 128 * kt - q0)
                            nq = QH - q_off
                            # segment [q_off, QH) at 512 boundaries: a single
                            # matmul's output must stay within one PSUM bank
                            segs = []
                            a = q_off
                            while a < QH:
                                b_end = min((a // 512 + 1) * 512, QH)
                                segs.append((a, b_end))
                                a = b_end
                            # absolute column indexing: tile col j = q0 + j;
                            # cols [0, q_off) stay unused so every matmul
                            # output is 512-bank-aligned
                            s_ps = spool.tile([128, QH], F32, tag="spsum")
                            for a, e in segs:
                                nc.tensor.matmul(
                                    s_ps[:, a:e],
                                    kwT_sb[
                                        poff : poff + 64, dt, kt * 128 : (kt + 1) * 128
                                    ],
                                    qwT_sb[poff : poff + 64, dt, q0 + a : q0 + e],
                                    start=True,
                                    stop=True,
                                )
                            if 128 * kt >= q0:
                                # diagonal block: causal bias
                                nc.vector.tensor_add(
                                    s_ps[:, q_off : q_off + 128],
                                    s_ps[:, q_off : q_off + 128],
                                    ctile_sb,
                                )
                            p_t = ppool.tile([128, QH], BF16, tag="p")
                            nc.scalar.activation(
                                p_t[:, q_off:QH],
                                s_ps[:, q_off:QH],
                                mybir.ActivationFunctionType.Exp,
                                bias=vbias_sb[:, kt : kt + 1],
                                scale=0.125,
                            )
                            for a, e in segs:
                                nc.tensor.matmul(
                                    out_ps[:, a:e],
                                    vw_sb[:, kt, h, :],
                                    p_t[:, a:e],
                                    start=(kt == 0),
                                    stop=(kt == last_kt),
                                )
                        ost = ostage.tile([65, QH], F32, tag="ost")
                        nc.vector.tensor_copy(ost, out_ps)
                        nc.sync.dma_start(
                            out=outT[h * 65 : (h + 1) * 65, q0 : q0 + QH], in_=ost
                        )
    nc.finalize()
    return nc


def _core_inputs(q, k, v, v_mask, Wq, Wk, Wv, b, hg):
    cols = slice(hg * HG_COLS, (hg + 1) * HG_COLS)
    vb = (NEG * (1.0 - v_mask[b])).astype(np.float32)
    ct = np.where(
        np.arange(128)[:, None] <= np.arange(128)[None, :], 0.0, NEG
    ).astype(np.float32)
    return {
        "xqT": np.ascontiguousarray(q[b].T),
        "xkT": np.ascontiguousarray(k[b].T),
        "xvT": np.ascontiguousarray(v[b].T),
        "wq": np.ascontiguousarray(Wq[:, cols]),
        "wk": np.ascontiguousarray(Wk[:, cols]),
        "wv": np.ascontiguousarray(Wv[:, cols]),
        "vbias": np.ascontiguousarray(vb.reshape(NKT, 128).T),
        "ctile": ct,
    }


def kernel(q, k, v, v_mask, q_mask, Wq, Wk, Wv):
    global LAST_RESULT, _NC_CACHE
    q = np.asarray(q, np.float32)
    k = np.asarray(k, np.float32)
    v = np.asarray(v, np.float32)
    v_mask = np.asarray(v_mask, np.float32)
    q_mask = np.asarray(q_mask, np.float32)
    Wq = np.asarray(Wq, np.float32)
    Wk = np.asarray(Wk, np.float32)
    Wv = np.asarray(Wv, np.float32)

    if _NC_CACHE is None:
        _NC_CACHE = _build_nc()
    nc = _NC_CACHE

    in_maps = [
        _core_inputs(q, k, v, v_mask, Wq, Wk, Wv, c // 2, c % 2) for c in range(8)
    ]
    _ensure_ntff_hook()
    res = run_bass_kernel_spmd(nc, in_maps, core_ids=list(range(8)))
    LAST_RESULT = res

    out = np.empty((B, S, D), np.float32)
    for c in range(8):
        b, hg = c // 2, c % 2
        o = np.asarray(res.results[c]["outT"], np.float32)  # [520, 2048]
        for h in range(HEADS_PER_CORE):
            pv = o[h * 65 : h * 65 + 64, :]  # [64, S]
            sm = o[h * 65 + 64, :]  # [S]
            sm = np.where(sm == 0.0, 1.0, sm)
            g = hg * HEADS_PER_CORE + h
            out[b, :, g * 64 : (g + 1) * 64] = (pv / sm).T
    out *= q_mask[:, :, None]

    # Degenerate rows: every causally-visible key masked. The reference's
    # additive -1e10 masks then make softmax uniform over all keys with
    # v_mask=1 (causality ignored). Patch on host; never triggers unless
    # v_mask[b, 0] == 0.
    for b in range(B):
        n_pref = int(np.argmax(v_mask[b] > 0)) if v_mask[b].max() > 0 else S
        if v_mask[b, 0] == 0 and n_pref > 0:
            vw_avg = ((v_mask[b] @ v[b]) / v_mask[b].sum()) @ Wv  # [OUT_DIM]
            out[b, :n_pref, :] = vw_avg[None, :] * q_mask[b, :n_pref, None]
    return out


# revision 10
# speedup vs baseline: 1.6370x; 1.6370x over previous
"""Masked causal multi-head attention on 8 TRN2 NeuronCores.

Problem (hardcoded shapes): B=4, S=2048, D_MODEL=1024, HEADS=16,
KEY_SIZE=SIZE_PER_HEAD=64, OUT_DIM=1024, fp32 I/O.

Sharding: pure data/tensor parallel — core c handles batch b=c//2 and
head-group hg=c%2 (8 heads). Each core's output shard [2048, 512] is
independent, so there are no collectives; the host assembles shards.

Per-core pipeline (all matmul inputs bf16 — fp32 matmuls lower to two
HW passes on trn2, so bf16 doubles PE throughput; PSUM accumulation is
fp32 either way):
  phase 1: DMA W shards (bf16) + mask-bias constants into SBUF.
  phase 2: projections. qw^T/kw^T stored [dh, s] (bf16), vw stored
           [s, dh] (bf16) with a fused ones-column per head (65th col)
           so the PV matmul also produces softmax denominators.
  phase 3: attention, heads processed in even/odd PAIRS so the PE
           always has one head's score/PV matmuls to run while the
           ScalarE computes the other head's exp — keeps the PE HAM
           clock warm. Per (pair, q-half, k-tile): scores S^T[k, q]
           (contraction over dh=64; even head in PE row group 0, odd
           head in row group 64 — concurrent), causal bias add on the
           diagonal block, exp((S + causal)/8 + key_bias) on ScalarE
           (no max-subtraction: masked lanes underflow to exactly 0,
           matching the reference's -1e10 additive masks), PV matmul
           accumulates out^T [65, q] in PSUM (row 64 = sum of exp via
           the ones-column).
  host:    divide by sums, multiply q_mask, transpose, assemble.
"""

import os
import sys

import numpy as np

for _p in ("/opt/trn_rl_repo",):
    if _p not in sys.path and os.path.isdir(_p):
        sys.path.insert(0, _p)

import ml_dtypes

import concourse.bass as bass
import concourse.mybir as mybir
import concourse.tile as tile
from concourse import bacc
from concourse.bass_utils import run_bass_kernel_spmd

B = 4
S = 2048
D = 1024
HEADS_PER_CORE = 8
DH = 64
HG_COLS = HEADS_PER_CORE * DH  # 512 output cols per core
NKT = S // 128  # 16 k-tiles
NEG = -1.0e9

F32 = mybir.dt.float32
BF16 = mybir.dt.bfloat16
NP_BF16 = ml_dtypes.bfloat16

LAST_RESULT = None  # stashed BassKernelResults for test harness inspection
_NC_CACHE = None


def _ensure_ntff_hook():
    """The agent image's antenv lacks axon_hooks; synthesize it so
    run_bass_kernel_spmd(trace=True) can reach the NTFF profiler."""
    try:
        from antenv.axon_hooks import get_axon_ntff_profile_hook  # noqa: F401

        return
    except ImportError:
        pass
    import types

    try:
        import antenv
        from trn_agent_boot.trn_boot import _ntff_profile_via_ctypes
    except ImportError:
        return
    mod = types.ModuleType("antenv.axon_hooks")
    _hook = [None]
    try:
        _hook[0] = _ntff_profile_via_ctypes("/opt/axon/libaxon_pjrt.so")
    except OSError:
        pass
    mod.set_axon_ntff_profile_hook = lambda h: _hook.__setitem__(0, h)
    mod.get_axon_ntff_profile_hook = lambda: _hook[0]
    sys.modules["antenv.axon_hooks"] = mod
    antenv.axon_hooks = mod


def _bank_segs(q_off, qh_width):
    """Segment [q_off, qh_width) at 512 boundaries: a matmul's PSUM
    output must stay within one bank."""
    segs = []
    a = q_off
    while a < qh_width:
        e = min((a // 512 + 1) * 512, qh_width)
        segs.append((a, e))
        a = e
    return segs


def _build_nc() -> bass.Bass:
    nc = bacc.Bacc()

    xqT = nc.declare_dram_parameter("xqT", [D, S], BF16, isOutput=False)[:]
    xkT = nc.declare_dram_parameter("xkT", [D, S], BF16, isOutput=False)[:]
    xvT = nc.declare_dram_parameter("xvT", [D, S], BF16, isOutput=False)[:]
    wq = nc.declare_dram_parameter("wq", [D, HG_COLS], BF16, isOutput=False)[:]
    wk = nc.declare_dram_parameter("wk", [D, HG_COLS], BF16, isOutput=False)[:]
    wv = nc.declare_dram_parameter("wv", [D, HG_COLS], BF16, isOutput=False)[:]
    vbias = nc.declare_dram_parameter("vbias", [128, NKT], F32, isOutput=False)[:]
    ctile = nc.declare_dram_parameter("ctile", [128, 128], F32, isOutput=False)[:]
    outT = nc.declare_dram_parameter(
        "outT", [HEADS_PER_CORE * 65, S], F32, isOutput=True
    )[:]

    with tile.TileContext(nc) as tc:
        with (
            tc.tile_pool(name="consts", bufs=1) as consts,
            tc.tile_pool(name="qk_sb", bufs=1) as qk_sb,
            tc.tile_pool(name="vw_pool", bufs=1) as vw_pool,
        ):
            vbias_sb = consts.tile([128, NKT], F32)
            nc.sync.dma_start(out=vbias_sb, in_=vbias)
            ctile_sb = consts.tile([128, 128], F32)
            nc.sync.dma_start(out=ctile_sb, in_=ctile)

            # projected tensors, resident for the whole attention phase
            qwT_sb = qk_sb.tile([128, 4, S], BF16)  # [dh%128, dh//128, s]
            kwT_sb = qk_sb.tile([128, 4, S], BF16)
            # vw with ones column: [s%128, s//128, head, 65]
            vw_sb = vw_pool.tile([128, NKT, HEADS_PER_CORE, 65], BF16)
            nc.vector.memset(vw_sb[:, :, :, 64:65], 1.0)

            # ---------------- phase 1+2: projections ----------------
            with (
                tc.tile_pool(name="wpool", bufs=1) as wpool,
                tc.tile_pool(name="xpool", bufs=2) as xpool,
                tc.tile_pool(name="pq", bufs=2, space="PSUM") as pqp,
                tc.tile_pool(name="pv", bufs=2, space="PSUM") as pvp,
            ):
                wq_sb = wpool.tile([128, 8, HG_COLS], BF16, tag="wq")
                nc.sync.dma_start(out=wq_sb, in_=wq.rearrange("(t p) n -> p t n", p=128))
                wk_sb = wpool.tile([128, 8, HG_COLS], BF16, tag="wk")
                nc.sync.dma_start(out=wk_sb, in_=wk.rearrange("(t p) n -> p t n", p=128))
                wv_sb = wpool.tile([128, 8, HG_COLS], BF16, tag="wv")
                nc.sync.dma_start(out=wv_sb, in_=wv.rearrange("(t p) n -> p t n", p=128))

                xqT_r = xqT.rearrange("(t p) s -> p t s", p=128)
                xkT_r = xkT.rearrange("(t p) s -> p t s", p=128)
                xvT_r = xvT.rearrange("(t p) s -> p t s", p=128)

                SC = 512  # s-chunk width
                for sc in range(S // SC):
                    ssl = slice(sc * SC, (sc + 1) * SC)
                    xq_t = xpool.tile([128, 8, SC], BF16, tag="xq")
                    nc.sync.dma_start(out=xq_t, in_=xqT_r[:, :, ssl])
                    xk_t = xpool.tile([128, 8, SC], BF16, tag="xk")
                    nc.sync.dma_start(out=xk_t, in_=xkT_r[:, :, ssl])
                    xv_t = xpool.tile([128, 8, SC], BF16, tag="xv")
                    nc.sync.dma_start(out=xv_t, in_=xvT_r[:, :, ssl])

                    # qw^T / kw^T: out[dh_tile, s] = W.T @ x^T
                    for w_sb, x_t, dst in (
                        (wq_sb, xq_t, qwT_sb),
                        (wk_sb, xk_t, kwT_sb),
                    ):
                        for dt in range(4):
                            ps = pqp.tile([128, SC], F32, tag="pq")
                            for t in range(8):
                                nc.tensor.matmul(
                                    ps,
                                    w_sb[:, t, dt * 128 : (dt + 1) * 128],
                                    x_t[:, t, :],
                                    start=(t == 0),
                                    stop=(t == 7),
                                )
                            nc.vector.tensor_copy(dst[:, dt, ssl], ps)

                    # vw: out[s_tile, dh] = x @ Wv  (natural layout)
                    for st2 in range(SC // 128):
                        kt = sc * (SC // 128) + st2
                        ps = pvp.tile([128, HG_COLS], F32, tag="pv")
                        for t in range(8):
                            nc.tensor.matmul(
                                ps,
                                xv_t[:, t, st2 * 128 : (st2 + 1) * 128],
                                wv_sb[:, t, :],
                                start=(t == 0),
                                stop=(t == 7),
                            )
                        nc.vector.tensor_copy(
                            vw_sb[:, kt, :, 0:64],
                            ps.rearrange("p (h d) -> p h d", h=HEADS_PER_CORE),
                        )

            # ---------------- phase 3: attention (head pairs) ----------------
            with (
                tc.tile_pool(name="ppool", bufs=4) as ppool,
                tc.tile_pool(name="ostage", bufs=2) as ostage,
                tc.tile_pool(name="att_se", bufs=1, space="PSUM") as spool_e,
                tc.tile_pool(name="att_so", bufs=1, space="PSUM") as spool_o,
                tc.tile_pool(name="att_oe", bufs=1, space="PSUM") as opool_e,
                tc.tile_pool(name="att_oo", bufs=1, space="PSUM") as opool_o,
            ):
                QH = 1024  # q processed in halves
                for pair in range(HEADS_PER_CORE // 2):
                    h_e, h_o = 2 * pair, 2 * pair + 1
                    dt = pair  # == h//2 for both heads
                    for qh in range(S // QH):
                        q0 = qh * QH
                        out_e = opool_e.tile([65, QH], F32, tag="oute")
                        out_o = opool_o.tile([65, QH], F32, tag="outo")
                        last_kt = (q0 + QH) // 128 - 1
                        for kt in range(last_kt + 1):
                            q_off = max(0, 128 * kt - q0)
                            segs = _bank_segs(q_off, QH)
                            work = []  # (head, poff, s_tile, out_tile)
                            s_e = spool_e.tile([128, QH], F32, tag="se")
                            s_o = spool_o.tile([128, QH], F32, tag="so")
                            work.append((h_e, 0, s_e, out_e))
                            work.append((h_o, 64, s_o, out_o))
                            # scores for both heads first (concurrent row
                            # groups 0 / 64 on the PE)
                            for h, poff, s_ps, _ in work:
                                for a, e in segs:
                                    nc.tensor.matmul(
                                        s_ps[:, a:e],
                                        kwT_sb[
                                            poff : poff + 64,
                                            dt,
                                            kt * 128 : (kt + 1) * 128,
                                        ],
                                        qwT_sb[poff : poff + 64, dt, q0 + a : q0 + e],
                                        start=True,
                                        stop=True,
                                    )
                            diag = 128 * kt >= q0
                            for h, poff, s_ps, out_ps in work:
                                if diag:
                                    nc.vector.tensor_add(
                                        s_ps[:, q_off : q_off + 128],
                                        s_ps[:, q_off : q_off + 128],
                                        ctile_sb,
                                    )
                                p_t = ppool.tile([128, QH], BF16, tag="p")
                                nc.scalar.activation(
                                    p_t[:, q_off:QH],
                                    s_ps[:, q_off:QH],
                                    mybir.ActivationFunctionType.Exp,
                                    bias=vbias_sb[:, kt : kt + 1],
                                    scale=0.125,
                                )
                                for a, e in segs:
                                    nc.tensor.matmul(
                                        out_ps[:, a:e],
                                        vw_sb[:, kt, h, :],
                                        p_t[:, a:e],
                                        start=(kt == 0),
                                        stop=(kt == last_kt),
                                    )
                        for h, out_ps in ((h_e, out_e), (h_o, out_o)):
                            ost = ostage.tile([65, QH], F32, tag="ost")
                            nc.vector.tensor_copy(ost, out_ps)
                            nc.sync.dma_start(
                                out=outT[h * 65 : (h + 1) * 65, q0 : q0 + QH], in_=ost
                            )
    nc.finalize()
    return nc


def _core_inputs(q, k, v, v_mask, Wq, Wk, Wv, b, hg):
    cols = slice(hg * HG_COLS, (hg + 1) * HG_COLS)
    vb = (NEG * (1.0 - v_mask[b])).astype(np.float32)
    ct = np.where(
        np.arange(128)[:, None] <= np.arange(128)[None, :], 0.0, NEG
    ).astype(np.float32)
    return {
        "xqT": np.ascontiguousarray(q[b].T).astype(NP_BF16),
        "xkT": np.ascontiguousarray(k[b].T).astype(NP_BF16),
        "xvT": np.ascontiguousarray(v[b].T).astype(NP_BF16),
        "wq": np.ascontiguousarray(Wq[:, cols]).astype(NP_BF16),
        "wk": np.ascontiguousarray(Wk[:, cols]).astype(NP_BF16),
        "wv": np.ascontiguousarray(Wv[:, cols]).astype(NP_BF16),
        "vbias": np.ascontiguousarray(vb.reshape(NKT, 128).T),
        "ctile": ct,
    }


def kernel(q, k, v, v_mask, q_mask, Wq, Wk, Wv):
    global LAST_RESULT, _NC_CACHE
    q = np.asarray(q, np.float32)
    k = np.asarray(k, np.float32)
    v = np.asarray(v, np.float32)
    v_mask = np.asarray(v_mask, np.float32)
    q_mask = np.asarray(q_mask, np.float32)
    Wq = np.asarray(Wq, np.float32)
    Wk = np.asarray(Wk, np.float32)
    Wv = np.asarray(Wv, np.float32)

    if _NC_CACHE is None:
        _NC_CACHE = _build_nc()
    nc = _NC_CACHE

    in_maps = [
        _core_inputs(q, k, v, v_mask, Wq, Wk, Wv, c // 2, c % 2) for c in range(8)
    ]
    _ensure_ntff_hook()
    res = run_bass_kernel_spmd(nc, in_maps, core_ids=list(range(8)))
    LAST_RESULT = res

    out = np.empty((B, S, D), np.float32)
    for c in range(8):
        b, hg = c // 2, c % 2
        o = np.asarray(res.results[c]["outT"], np.float32)  # [520, 2048]
        for h in range(HEADS_PER_CORE):
            pv = o[h * 65 : h * 65 + 64, :]  # [64, S]
            sm = o[h * 65 + 64, :]  # [S]
            sm = np.where(sm == 0.0, 1.0, sm)
            g = hg * HEADS_PER_CORE + h
            out[b, :, g * 64 : (g + 1) * 64] = (pv / sm).T
    out *= q_mask[:, :, None]

    # Degenerate rows: every causally-visible key masked. The reference's
    # additive -1e10 masks then make softmax uniform over all keys with
    # v_mask=1 (causality ignored). Patch on host; never triggers unless
    # v_mask[b, 0] == 0.
    for b in range(B):
        n_pref = int(np.argmax(v_mask[b] > 0)) if v_mask[b].max() > 0 else S
        if v_mask[b, 0] == 0 and n_pref > 0:
            vw_avg = ((v_mask[b] @ v[b]) / v_mask[b].sum()) @ Wv  # [OUT_DIM]
            out[b, :n_pref, :] = vw_avg[None, :] * q_mask[b, :n_pref, None]
    return out


# revision 11
# speedup vs baseline: 1.9270x; 1.1771x over previous
"""Masked causal multi-head attention on 8 TRN2 NeuronCores.

Problem (hardcoded shapes): B=4, S=2048, D_MODEL=1024, HEADS=16,
KEY_SIZE=SIZE_PER_HEAD=64, OUT_DIM=1024, fp32 I/O.

Sharding: pure data/tensor parallel — core c handles batch b=c//2 and
head-group hg=c%2 (8 heads). Each core's output shard [2048, 512] is
independent, so there are no collectives; the host assembles shards.

Per-core pipeline (all matmul inputs bf16 — fp32 matmuls lower to two
HW passes on trn2, so bf16 doubles PE throughput; PSUM accumulation is
fp32 either way):
  - x^T for q,k kept SBUF-resident; per head-pair dt: project
    qw^T/kw^T[dt] right before that pair's attention, so the Tile
    scheduler overlaps pair p+1's projection matmuls with pair p's
    exp-paced attention (keeps the PE dense -> HAM clock stays warm).
  - vw [s, dh] (bf16) carries a fused ones-column per head (65th col)
    so the PV matmul also emits softmax denominators.
  - attention per (pair, q-chunk 512, k-tile): scores S^T[k, q] with
    even head on PE rows 0-63 and odd head on rows 64-127 (concurrent
    row groups); exp((S)/8 + key_bias) on ScalarE with NO
    max-subtraction (masked lanes underflow to exactly 0, matching
    the reference's -1e10 additive masks); causal masking of the
    diagonal block is a 0/1 multiply on P AFTER exp (off ScalarE's
    critical path); PV accumulates out^T [65, q] in PSUM.
  - every PSUM tile is exactly one 512-f32 bank: S pool bufs=4 +
    out_e + out_o + projection pool = 8 banks.
  host: divide by sums, multiply q_mask, transpose, assemble.
"""

import os
import sys

import numpy as np

for _p in ("/opt/trn_rl_repo",):
    if _p not in sys.path and os.path.isdir(_p):
        sys.path.insert(0, _p)

import ml_dtypes

import concourse.bass as bass
import concourse.mybir as mybir
import concourse.tile as tile
from concourse import bacc
from concourse.bass_utils import run_bass_kernel_spmd

B = 4
S = 2048
D = 1024
HEADS_PER_CORE = 8
DH = 64
HG_COLS = HEADS_PER_CORE * DH  # 512 output cols per core
NKT = S // 128  # 16 k-tiles
NEG = -1.0e9
QC = 512  # q-chunk width (one PSUM bank)

F32 = mybir.dt.float32
BF16 = mybir.dt.bfloat16
NP_BF16 = ml_dtypes.bfloat16

LAST_RESULT = None  # stashed BassKernelResults for test harness inspection
_NC_CACHE = None


def _ensure_ntff_hook():
    """The agent image's antenv lacks axon_hooks; synthesize it so
    run_bass_kernel_spmd(trace=True) can reach the NTFF profiler."""
    try:
        from antenv.axon_hooks import get_axon_ntff_profile_hook  # noqa: F401

        return
    except ImportError:
        pass
    import types

    try:
        import antenv
        from trn_agent_boot.trn_boot import _ntff_profile_via_ctypes
    except ImportError:
        return
    mod = types.ModuleType("antenv.axon_hooks")
    _hook = [None]
    try:
        _hook[0] = _ntff_profile_via_ctypes("/opt/axon/libaxon_pjrt.so")
    except OSError:
        pass
    mod.set_axon_ntff_profile_hook = lambda h: _hook.__setitem__(0, h)
    mod.get_axon_ntff_profile_hook = lambda: _hook[0]
    sys.modules["antenv.axon_hooks"] = mod
    antenv.axon_hooks = mod


def _build_nc() -> bass.Bass:
    nc = bacc.Bacc()

    xqT = nc.declare_dram_parameter("xqT", [D, S], BF16, isOutput=False)[:]
    xkT = nc.declare_dram_parameter("xkT", [D, S], BF16, isOutput=False)[:]
    xvT = nc.declare_dram_parameter("xvT", [D, S], BF16, isOutput=False)[:]
    wq = nc.declare_dram_parameter("wq", [D, HG_COLS], BF16, isOutput=False)[:]
    wk = nc.declare_dram_parameter("wk", [D, HG_COLS], BF16, isOutput=False)[:]
    wv = nc.declare_dram_parameter("wv", [D, HG_COLS], BF16, isOutput=False)[:]
    vbias = nc.declare_dram_parameter("vbias", [128, NKT], F32, isOutput=False)[:]
    ctile = nc.declare_dram_parameter("ctile", [128, 128], BF16, isOutput=False)[:]
    outT = nc.declare_dram_parameter(
        "outT", [HEADS_PER_CORE * 65, S], F32, isOutput=True
    )[:]

    with tile.TileContext(nc) as tc:
        with (
            tc.tile_pool(name="consts", bufs=1) as consts,
            tc.tile_pool(name="xqk", bufs=1) as xqk,
            tc.tile_pool(name="wpool", bufs=1) as wpool,
            tc.tile_pool(name="qk_sb", bufs=1) as qk_sb,
            tc.tile_pool(name="vw_pool", bufs=1) as vw_pool,
            tc.tile_pool(name="xvpool", bufs=2) as xvpool,
            tc.tile_pool(name="ppool", bufs=6) as ppool,
            tc.tile_pool(name="ostage", bufs=4) as ostage,
            tc.tile_pool(name="proj_ps", bufs=2, space="PSUM") as proj_ps,
            tc.tile_pool(name="att_s", bufs=4, space="PSUM") as spool,
            tc.tile_pool(name="att_oe", bufs=1, space="PSUM") as opool_e,
            tc.tile_pool(name="att_oo", bufs=1, space="PSUM") as opool_o,
        ):
            vbias_sb = consts.tile([128, NKT], F32)
            nc.sync.dma_start(out=vbias_sb, in_=vbias)
            ctile_sb = consts.tile([128, 128], BF16)
            nc.sync.dma_start(out=ctile_sb, in_=ctile)

            wq_sb = wpool.tile([128, 8, HG_COLS], BF16, tag="wq")
            nc.sync.dma_start(out=wq_sb, in_=wq.rearrange("(t p) n -> p t n", p=128))
            wk_sb = wpool.tile([128, 8, HG_COLS], BF16, tag="wk")
            nc.sync.dma_start(out=wk_sb, in_=wk.rearrange("(t p) n -> p t n", p=128))
            wv_sb = wpool.tile([128, 8, HG_COLS], BF16, tag="wv")
            nc.sync.dma_start(out=wv_sb, in_=wv.rearrange("(t p) n -> p t n", p=128))

            # x^T for q and k fully resident (bf16): 32 KB/partition each
            xq_sb = xqk.tile([128, 8, S], BF16, tag="xq")
            nc.sync.dma_start(out=xq_sb, in_=xqT.rearrange("(t p) s -> p t s", p=128))
            xk_sb = xqk.tile([128, 8, S], BF16, tag="xk")
            nc.sync.dma_start(out=xk_sb, in_=xkT.rearrange("(t p) s -> p t s", p=128))

            # projected tensors
            qwT_sb = qk_sb.tile([128, 4, S], BF16)  # [dh%128, dh//128, s]
            kwT_sb = qk_sb.tile([128, 4, S], BF16)
            vw_sb = vw_pool.tile([128, NKT, HEADS_PER_CORE, 65], BF16)
            nc.vector.memset(vw_sb[:, :, :, 64:65], 1.0)

            # ---- vw projection (streams x_v) ----
            xvT_r = xvT.rearrange("(t p) s -> p t s", p=128)
            for sc in range(S // 512):
                xv_t = xvpool.tile([128, 8, 512], BF16, tag="xv")
                nc.sync.dma_start(
                    out=xv_t, in_=xvT_r[:, :, sc * 512 : (sc + 1) * 512]
                )
                for st2 in range(4):
                    kt = sc * 4 + st2
                    ps = proj_ps.tile([128, HG_COLS], F32, tag="pp")
                    for t in range(8):
                        nc.tensor.matmul(
                            ps,
                            xv_t[:, t, st2 * 128 : (st2 + 1) * 128],
                            wv_sb[:, t, :],
                            start=(t == 0),
                            stop=(t == 7),
                        )
                    nc.vector.tensor_copy(
                        vw_sb[:, kt, :, 0:64],
                        ps.rearrange("p (h d) -> p h d", h=HEADS_PER_CORE),
                    )

            # ---- per head-pair: project qw^T/kw^T[dt], then attention ----
            for pair in range(HEADS_PER_CORE // 2):
                dt = pair
                h_e, h_o = 2 * pair, 2 * pair + 1

                for w_sb, x_sb, dst in (
                    (wq_sb, xq_sb, qwT_sb),
                    (wk_sb, xk_sb, kwT_sb),
                ):
                    for sc in range(S // 512):
                        ps = proj_ps.tile([128, 512], F32, tag="pp")
                        for t in range(8):
                            nc.tensor.matmul(
                                ps,
                                w_sb[:, t, dt * 128 : (dt + 1) * 128],
                                x_sb[:, t, sc * 512 : (sc + 1) * 512],
                                start=(t == 0),
                                stop=(t == 7),
                            )
                        nc.vector.tensor_copy(
                            dst[:, dt, sc * 512 : (sc + 1) * 512], ps
                        )

                for qc in range(S // QC):
                    q0 = qc * QC
                    out_e = opool_e.tile([65, QC], F32, tag="oute")
                    out_o = opool_o.tile([65, QC], F32, tag="outo")
                    last_kt = (q0 + QC) // 128 - 1
                    for kt in range(last_kt + 1):
                        q_off = max(0, 128 * kt - q0)
                        s_e = spool.tile([128, QC], F32, tag="s")
                        s_o = spool.tile([128, QC], F32, tag="s")
                        work = ((0, s_e, out_e, h_e), (64, s_o, out_o, h_o))
                        # both heads' score matmuls back-to-back: they land
                        # on disjoint PE row groups (0 / 64) -> concurrent
                        for poff, s_ps, _, _ in work:
                            nc.tensor.matmul(
                                s_ps[:, q_off:QC],
                                kwT_sb[
                                    poff : poff + 64, dt, kt * 128 : (kt + 1) * 128
                                ],
                                qwT_sb[poff : poff + 64, dt, q0 + q_off : q0 + QC],
                                start=True,
                                stop=True,
                            )
                        diag = 128 * kt >= q0
                        for poff, s_ps, out_ps, h in work:
                            p_t = ppool.tile([128, QC], BF16, tag="p")
                            nc.scalar.activation(
                                p_t[:, q_off:QC],
                                s_ps[:, q_off:QC],
                                mybir.ActivationFunctionType.Exp,
                                bias=vbias_sb[:, kt : kt + 1],
                                scale=0.125,
                            )
                            if diag:
                                # causal mask as 0/1 multiply AFTER exp
                                nc.vector.tensor_mul(
                                    p_t[:, q_off : q_off + 128],
                                    p_t[:, q_off : q_off + 128],
                                    ctile_sb,
                                )
                            nc.tensor.matmul(
                                out_ps[:, q_off:QC],
                                vw_sb[:, kt, h, :],
                                p_t[:, q_off:QC],
                                start=(kt == 0),
                                stop=(kt == last_kt),
                            )
                    for h, out_ps in ((h_e, out_e), (h_o, out_o)):
                        ost = ostage.tile([65, QC], F32, tag="ost")
                        nc.vector.tensor_copy(ost, out_ps)
                        nc.sync.dma_start(
                            out=outT[h * 65 : (h + 1) * 65, q0 : q0 + QC], in_=ost
                        )
    nc.finalize()
    return nc


def _core_inputs(q, k, v, v_mask, Wq, Wk, Wv, b, hg):
    cols = slice(hg * HG_COLS, (hg + 1) * HG_COLS)
    vb = (NEG * (1.0 - v_mask[b])).astype(np.float32)
    # causal 0/1 keep-mask for the diagonal block of S^T[k, q]: keep k <= q
    ct = (np.arange(128)[:, None] <= np.arange(128)[None, :]).astype(NP_BF16)
    return {
        "xqT": np.ascontiguousarray(q[b].T).astype(NP_BF16),
        "xkT": np.ascontiguousarray(k[b].T).astype(NP_BF16),
        "xvT": np.ascontiguousarray(v[b].T).astype(NP_BF16),
        "wq": np.ascontiguousarray(Wq[:, cols]).astype(NP_BF16),
        "wk": np.ascontiguousarray(Wk[:, cols]).astype(NP_BF16),
        "wv": np.ascontiguousarray(Wv[:, cols]).astype(NP_BF16),
        "vbias": np.ascontiguousarray(vb.reshape(NKT, 128).T),
        "ctile": ct,
    }


def kernel(q, k, v, v_mask, q_mask, Wq, Wk, Wv):
    global LAST_RESULT, _NC_CACHE
    q = np.asarray(q, np.float32)
    k = np.asarray(k, np.float32)
    v = np.asarray(v, np.float32)
    v_mask = np.asarray(v_mask, np.float32)
    q_mask = np.asarray(q_mask, np.float32)
    Wq = np.asarray(Wq, np.float32)
    Wk = np.asarray(Wk, np.float32)
    Wv = np.asarray(Wv, np.float32)

    if _NC_CACHE is None:
        _NC_CACHE = _build_nc()
    nc = _NC_CACHE

    in_maps = [
        _core_inputs(q, k, v, v_mask, Wq, Wk, Wv, c // 2, c % 2) for c in range(8)
    ]
    _ensure_ntff_hook()
    res = run_bass_kernel_spmd(nc, in_maps, core_ids=list(range(8)))
    LAST_RESULT = res

    out = np.empty((B, S, D), np.float32)
    for c in range(8):
        b, hg = c // 2, c % 2
        o = np.asarray(res.results[c]["outT"], np.float32)  # [520, 2048]
        for h in range(HEADS_PER_CORE):
            pv = o[h * 65 : h * 65 + 64, :]  # [64, S]
            sm = o[h * 65 + 64, :]  # [S]
            sm = np.where(sm == 0.0, 1.0, sm)
            g = hg * HEADS_PER_CORE + h
            out[b, :, g * 64 : (g + 1) * 64] = (pv / sm).T
    out *= q_mask[:, :, None]

    # Degenerate rows: every causally-visible key masked. The reference's
    # additive -1e10 masks then make softmax uniform over all keys with
    # v_mask=1 (causality ignored). Patch on host; never triggers unless
    # v_mask[b, 0] == 0.
    for b in range(B):
        n_pref = int(np.argmax(v_mask[b] > 0)) if v_mask[b].max() > 0 else S
        if v_mask[b, 0] == 0 and n_pref > 0:
            vw_avg = ((v_mask[b] @ v[b]) / v_mask[b].sum()) @ Wv  # [OUT_DIM]
            out[b, :n_pref, :] = vw_avg[None, :] * q_mask[b, :n_pref, None]
    return out


# revision 13
# speedup vs baseline: 2.2812x; 1.1838x over previous
"""Masked causal multi-head attention on 8 TRN2 NeuronCores.

Problem (hardcoded shapes): B=4, S=2048, D_MODEL=1024, HEADS=16,
KEY_SIZE=SIZE_PER_HEAD=64, OUT_DIM=1024, fp32 I/O.

Sharding: pure data/tensor parallel — core c handles batch b=c//2 and
head-group hg=c%2 (8 heads). Each core's output shard [2048, 512] is
independent, so there are no collectives; the host assembles shards.

Per-core pipeline (all matmul inputs bf16 — fp32 matmuls lower to two
HW passes on trn2, so bf16 doubles PE throughput; PSUM accumulation is
fp32 either way):
  - x^T for q,k kept SBUF-resident; per head-pair dt: project
    qw^T/kw^T[dt] right before that pair's attention, so the Tile
    scheduler overlaps pair p+1's projection matmuls with pair p's
    exp-paced attention (keeps the PE dense -> HAM clock stays warm).
  - vw [s, dh] (bf16) carries a fused ones-column per head (65th col)
    so the PV matmul also emits softmax denominators.
  - attention per (pair, q-chunk 512, k-tile): scores S^T[k, q] with
    even head on PE rows 0-63 and odd head on rows 64-127 (concurrent
    row groups); exp((S)/8 + key_bias) on ScalarE with NO
    max-subtraction (masked lanes underflow to exactly 0, matching
    the reference's -1e10 additive masks); causal masking of the
    diagonal block is a 0/1 multiply on P AFTER exp (off ScalarE's
    critical path); PV accumulates out^T [65, q] in PSUM.
  - every PSUM tile is exactly one 512-f32 bank: S pool bufs=4 +
    out_e + out_o + projection pool = 8 banks.
  host: divide by sums, multiply q_mask, transpose, assemble.
"""

import os
import sys

import numpy as np

for _p in ("/opt/trn_rl_repo",):
    if _p not in sys.path and os.path.isdir(_p):
        sys.path.insert(0, _p)

import ml_dtypes

import concourse.bass as bass
import concourse.mybir as mybir
import concourse.tile as tile
from concourse import bacc
from concourse.bass_utils import run_bass_kernel_spmd

B = 4
S = 2048
D = 1024
HEADS_PER_CORE = 8
DH = 64
HG_COLS = HEADS_PER_CORE * DH  # 512 output cols per core
NKT = S // 128  # 16 k-tiles
NEG = -1.0e9
QC = 512  # q-chunk width (one PSUM bank)

F32 = mybir.dt.float32
BF16 = mybir.dt.bfloat16
NP_BF16 = ml_dtypes.bfloat16

LAST_RESULT = None  # stashed BassKernelResults for test harness inspection
_NC_CACHE = None


def _ensure_ntff_hook():
    """The agent image's antenv lacks axon_hooks; synthesize it so
    run_bass_kernel_spmd(trace=True) can reach the NTFF profiler."""
    try:
        from antenv.axon_hooks import get_axon_ntff_profile_hook  # noqa: F401

        return
    except ImportError:
        pass
    import types

    try:
        import antenv
        from trn_agent_boot.trn_boot import _ntff_profile_via_ctypes
    except ImportError:
        return
    mod = types.ModuleType("antenv.axon_hooks")
    _hook = [None]
    try:
        _hook[0] = _ntff_profile_via_ctypes("/opt/axon/libaxon_pjrt.so")
    except OSError:
        pass
    mod.set_axon_ntff_profile_hook = lambda h: _hook.__setitem__(0, h)
    mod.get_axon_ntff_profile_hook = lambda: _hook[0]
    sys.modules["antenv.axon_hooks"] = mod
    antenv.axon_hooks = mod


def _build_nc() -> bass.Bass:
    nc = bacc.Bacc()

    xqT = nc.declare_dram_parameter("xqT", [D, S], BF16, isOutput=False)[:]
    xkT = nc.declare_dram_parameter("xkT", [D, S], BF16, isOutput=False)[:]
    xvT = nc.declare_dram_parameter("xvT", [D, S], BF16, isOutput=False)[:]
    wq = nc.declare_dram_parameter("wq", [D, HG_COLS], BF16, isOutput=False)[:]
    wk = nc.declare_dram_parameter("wk", [D, HG_COLS], BF16, isOutput=False)[:]
    wv = nc.declare_dram_parameter("wv", [D, HG_COLS], BF16, isOutput=False)[:]
    vbias = nc.declare_dram_parameter("vbias", [128, NKT], F32, isOutput=False)[:]
    ctile = nc.declare_dram_parameter("ctile", [128, 128], BF16, isOutput=False)[:]
    outT = nc.declare_dram_parameter(
        "outT", [HEADS_PER_CORE * 65, S], F32, isOutput=True
    )[:]

    with tile.TileContext(nc) as tc:
        with (
            tc.tile_pool(name="consts", bufs=1) as consts,
            tc.tile_pool(name="xqk", bufs=1) as xqk,
            tc.tile_pool(name="wpool", bufs=1) as wpool,
            tc.tile_pool(name="qk_sb", bufs=1) as qk_sb,
            tc.tile_pool(name="vw_pool", bufs=1) as vw_pool,
            tc.tile_pool(name="xvpool", bufs=2) as xvpool,
            tc.tile_pool(name="ppool", bufs=6) as ppool,
            tc.tile_pool(name="ostage", bufs=4) as ostage,
            tc.tile_pool(name="proj_ps", bufs=2, space="PSUM") as proj_ps,
            tc.tile_pool(name="att_s", bufs=4, space="PSUM") as spool,
            tc.tile_pool(name="att_oe", bufs=1, space="PSUM") as opool_e,
            tc.tile_pool(name="att_oo", bufs=1, space="PSUM") as opool_o,
        ):
            vbias_sb = consts.tile([128, NKT], F32)
            nc.sync.dma_start(out=vbias_sb, in_=vbias)
            ctile_sb = consts.tile([128, 128], BF16)
            nc.sync.dma_start(out=ctile_sb, in_=ctile)

            # DMA in need-order: vw projection inputs first, then q/k
            wv_sb = wpool.tile([128, 8, HG_COLS], BF16, tag="wv")
            nc.sync.dma_start(out=wv_sb, in_=wv.rearrange("(t p) n -> p t n", p=128))
            vw_sb = vw_pool.tile([128, NKT, HEADS_PER_CORE, 65], BF16)
            nc.vector.memset(vw_sb[:, :, :, 64:65], 1.0)

            xvT_r = xvT.rearrange("(t p) s -> p t s", p=128)
            xv_ts = []
            for sc in range(S // 512):
                xv_t = xvpool.tile([128, 8, 512], BF16, tag="xv")
                nc.sync.dma_start(
                    out=xv_t, in_=xvT_r[:, :, sc * 512 : (sc + 1) * 512]
                )
                xv_ts.append(xv_t)

            wq_sb = wpool.tile([128, 8, HG_COLS], BF16, tag="wq")
            nc.sync.dma_start(out=wq_sb, in_=wq.rearrange("(t p) n -> p t n", p=128))
            wk_sb = wpool.tile([128, 8, HG_COLS], BF16, tag="wk")
            nc.sync.dma_start(out=wk_sb, in_=wk.rearrange("(t p) n -> p t n", p=128))

            # x^T for q and k resident as per-512-chunk tiles (separate
            # tiles -> the first projections gate on one 1 MB DMA, not 8 MB)
            xqT_r = xqT.rearrange("(t p) s -> p t s", p=128)
            xkT_r = xkT.rearrange("(t p) s -> p t s", p=128)
            xq_cs, xk_cs = [], []
            for sc in range(S // 512):
                for src, lst, tg in ((xqT_r, xq_cs, "xq"), (xkT_r, xk_cs, "xk")):
                    t_ = xqk.tile([128, 8, 512], BF16, tag=f"{tg}{sc}")
                    nc.sync.dma_start(out=t_, in_=src[:, :, sc * 512 : (sc + 1) * 512])
                    lst.append(t_)

            # projected tensors
            qwT_sb = qk_sb.tile([128, 4, S], BF16)  # [dh%128, dh//128, s]
            kwT_sb = qk_sb.tile([128, 4, S], BF16)

            # ---- vw projection ----
            for sc in range(S // 512):
                for st2 in range(4):
                    kt = sc * 4 + st2
                    ps = proj_ps.tile([128, HG_COLS], F32, tag="pp")
                    for t in range(8):
                        nc.tensor.matmul(
                            ps,
                            xv_ts[sc][:, t, st2 * 128 : (st2 + 1) * 128],
                            wv_sb[:, t, :],
                            start=(t == 0),
                            stop=(t == 7),
                        )
                    nc.vector.tensor_copy(
                        vw_sb[:, kt, :, 0:64],
                        ps.rearrange("p (h d) -> p h d", h=HEADS_PER_CORE),
                    )

            # ---- per head-pair: project qw^T/kw^T[dt], then attention ----
            for pair in range(HEADS_PER_CORE // 2):
                dt = pair
                h_e, h_o = 2 * pair, 2 * pair + 1

                for w_sb, x_cs, dst in (
                    (wq_sb, xq_cs, qwT_sb),
                    (wk_sb, xk_cs, kwT_sb),
                ):
                    for sc in range(S // 512):
                        ps = proj_ps.tile([128, 512], F32, tag="pp")
                        for t in range(8):
                            nc.tensor.matmul(
                                ps,
                                w_sb[:, t, dt * 128 : (dt + 1) * 128],
                                x_cs[sc][:, t, :],
                                start=(t == 0),
                                stop=(t == 7),
                            )
                        nc.vector.tensor_copy(
                            dst[:, dt, sc * 512 : (sc + 1) * 512], ps
                        )

                for qc in range(S // QC):
                    q0 = qc * QC
                    out_e = opool_e.tile([65, QC], F32, tag="oute")
                    out_o = opool_o.tile([65, QC], F32, tag="outo")
                    last_kt = (q0 + QC) // 128 - 1
                    for kt in range(last_kt + 1):
                        q_off = max(0, 128 * kt - q0)
                        s_e = spool.tile([128, QC], F32, tag="s")
                        s_o = spool.tile([128, QC], F32, tag="s")
                        work = ((0, s_e, out_e, h_e), (64, s_o, out_o, h_o))
                        # both heads' score matmuls back-to-back: they land
                        # on disjoint PE row groups (0 / 64) -> concurrent
                        for poff, s_ps, _, _ in work:
                            nc.tensor.matmul(
                                s_ps[:, q_off:QC],
                                kwT_sb[
                                    poff : poff + 64, dt, kt * 128 : (kt + 1) * 128
                                ],
                                qwT_sb[poff : poff + 64, dt, q0 + q_off : q0 + QC],
                                start=True,
                                stop=True,
                            )
                        diag = 128 * kt >= q0
                        for poff, s_ps, out_ps, h in work:
                            p_t = ppool.tile([128, QC], BF16, tag="p")
                            nc.scalar.activation(
                                p_t[:, q_off:QC],
                                s_ps[:, q_off:QC],
                                mybir.ActivationFunctionType.Exp,
                                bias=vbias_sb[:, kt : kt + 1],
                                scale=0.125,
                            )
                            if diag:
                                # causal mask as 0/1 multiply AFTER exp
                                nc.vector.tensor_mul(
                                    p_t[:, q_off : q_off + 128],
                                    p_t[:, q_off : q_off + 128],
                                    ctile_sb,
                                )
                            nc.tensor.matmul(
                                out_ps[:, q_off:QC],
                                vw_sb[:, kt, h, :],
                                p_t[:, q_off:QC],
                                start=(kt == 0),
                                stop=(kt == last_kt),
                            )
                    for h, out_ps in ((h_e, out_e), (h_o, out_o)):
                        ost = ostage.tile([65, QC], F32, tag="ost")
                        nc.vector.tensor_copy(ost, out_ps)
                        nc.sync.dma_start(
                            out=outT[h * 65 : (h + 1) * 65, q0 : q0 + QC], in_=ost
                        )
    nc.finalize()
    return nc


def _core_inputs(q, k, v, v_mask, Wq, Wk, Wv, b, hg):
    cols = slice(hg * HG_COLS, (hg + 1) * HG_COLS)
    vb = (NEG * (1.0 - v_mask[b])).astype(np.float32)
    # causal 0/1 keep-mask for the diagonal block of S^T[k, q]: keep k <= q
    ct = (np.arange(128)[:, None] <= np.arange(128)[None, :]).astype(NP_BF16)
    return {
        "xqT": np.ascontiguousarray(q[b].T).astype(NP_BF16),
        "xkT": np.ascontiguousarray(k[b].T).astype(NP_BF16),
        "xvT": np.ascontiguousarray(v[b].T).astype(NP_BF16),
        "wq": np.ascontiguousarray(Wq[:, cols]).astype(NP_BF16),
        "wk": np.ascontiguousarray(Wk[:, cols]).astype(NP_BF16),
        "wv": np.ascontiguousarray(Wv[:, cols]).astype(NP_BF16),
        "vbias": np.ascontiguousarray(vb.reshape(NKT, 128).T),
        "ctile": ct,
    }


def kernel(q, k, v, v_mask, q_mask, Wq, Wk, Wv):
    global LAST_RESULT, _NC_CACHE
    q = np.asarray(q, np.float32)
    k = np.asarray(k, np.float32)
    v = np.asarray(v, np.float32)
    v_mask = np.asarray(v_mask, np.float32)
    q_mask = np.asarray(q_mask, np.float32)
    Wq = np.asarray(Wq, np.float32)
    Wk = np.asarray(Wk, np.float32)
    Wv = np.asarray(Wv, np.float32)

    if _NC_CACHE is None:
        _NC_CACHE = _build_nc()
    nc = _NC_CACHE

    in_maps = [
        _core_inputs(q, k, v, v_mask, Wq, Wk, Wv, c // 2, c % 2) for c in range(8)
    ]
    _ensure_ntff_hook()
    res = run_bass_kernel_spmd(nc, in_maps, core_ids=list(range(8)))
    LAST_RESULT = res

    out = np.empty((B, S, D), np.float32)
    for c in range(8):
        b, hg = c // 2, c % 2
        o = np.asarray(res.results[c]["outT"], np.float32)  # [520, 2048]
        for h in range(HEADS_PER_CORE):
            pv = o[h * 65 : h * 65 + 64, :]  # [64, S]
            sm = o[h * 65 + 64, :]  # [S]
            sm = np.where(sm == 0.0, 1.0, sm)
            g = hg * HEADS_PER_CORE + h
            out[b, :, g * 64 : (g + 1) * 64] = (pv / sm).T
    out *= q_mask[:, :, None]

    # Degenerate rows: every causally-visible key masked. The reference's
    # additive -1e10 masks then make softmax uniform over all keys with
    # v_mask=1 (causality ignored). Patch on host; never triggers unless
    # v_mask[b, 0] == 0.
    for b in range(B):
        n_pref = int(np.argmax(v_mask[b] > 0)) if v_mask[b].max() > 0 else S
        if v_mask[b, 0] == 0 and n_pref > 0:
            vw_avg = ((v_mask[b] @ v[b]) / v_mask[b].sum()) @ Wv  # [OUT_DIM]
            out[b, :n_pref, :] = vw_avg[None, :] * q_mask[b, :n_pref, None]
    return out
